# revision 11
# baseline (speedup 1.0000x reference)
"""BC6H surrogate block-level decode kernel for 8 Trainium2 NeuronCores.

Full-input contract: kernel(**inputs) takes the complete arrays from
setup_inputs() and returns the full (3, 4096, 4096) image.  Internally the
block dimension (nb = 1048576) is sharded 8 ways (pure data parallel); each
core runs an identical Bass/Tile program on its 131072-block shard.

Math (per 4x4 block b, pixel p in 0..15, channel c in 0..2), with
T_i = tanh(x/2) so sigmoid(x) = (1+T)/2 and every ACT call stays in the
exp_and_others table set (no ACT table reloads):
  w64   = 31.5*T_w + 31.5 + clip(3.5*T_w + 0.5, 0, 1)        (= 64*w, exact)
  num_p = sum_k exp(l_k) bank[k,p] ; den = sum_k exp(l_k)     (TensorE, bf16)
  u'    = s2u_c + bus_c*w64 + (cur_c + dur_c*w64)*num_p
          with rcp=1/den folded into cur/dur, and s2u folding
          -(1+1/1024+0.5) so u' is the magic-round floor input
  hh-14 = (u' + M) - (M+14),  M = 1.5*2^23                    (fp32 exact)
  out   = 2^(hh-14) * ((u'+1.5009765625) - ((u'+M)-M))        (one fused op)

Engine plan (per supertile of 4096 blocks, g=32 blocks/partition-row):
  DMA   : bf16 inputs (ep/ix/lg) in, bf16 blocks out
  ACT   : tanh(ep), tanh(ix), exp(logits^T) -> bf16, exp(hh-14)
  PE    : 128x128 bf16 transposes of logits; K=128 block-diag bank matmul
  DVE   : everything elementwise (GPSIMD is 2-4x slower per trace - unused)
"""

import sys

sys.path.insert(0, "/opt/trn_rl_repo")

from contextlib import ExitStack

import numpy as np
import ml_dtypes

import concourse.bass as bass
import concourse.tile as tile
from concourse import bacc, mybir
from concourse import bass_utils
from concourse import dve_ops
from concourse.dve_ops import DveOp
from concourse.dve_spec import (
    Spec,
    Src0,
    Src1,
    C0,
    C1,
    C2,
    One,
    relu,
    minn,
    lower,
    _has_src1,
)
from concourse.dve_uop import DveOpSpec

F32 = mybir.dt.float32
BF16 = mybir.dt.bfloat16
AOp = mybir.AluOpType
AF = mybir.ActivationFunctionType
BF_NP = ml_dtypes.bfloat16

# ---------------------------------------------------------------- constants
NB = 1048576
N_CORES = 8
NB_CORE = NB // N_CORES            # 131072 blocks per core
G = 32                             # blocks per partition-row per supertile
H = W = 4096
BY = BX = 1024

EU_SCALE = 31248.0 / 1024.0        # 30.515625
EU_BIAS = 248.0 / 1024.0           # 0.2421875
ES2 = EU_SCALE / 2.0               # tanh-domain endpoint scale
FLOOR_OFF_H = 1.0 + 1.0 / 1024.0 + 0.5   # 1.5009765625 (exact in f32)
MAGIC = 12582912.0                 # 1.5 * 2^23
LN2 = 0.6931471805599453

# ------------------------------------------------------- custom DVE ops
_REGISTERED = {}


def _register(name, spec):
    if name in _REGISTERED:
        return _REGISTERED[name]
    if name not in dve_ops._SUB_OPCODE_FOR_NAME:
        row = max(dve_ops._SUB_OPCODE_FOR_NAME.values()) + 1
        assert row < 0x20, "custom-DVE opcode rows exhausted"
        dve_ops._SUB_OPCODE_FOR_NAME[name] = row
    row = dve_ops._SUB_OPCODE_FOR_NAME[name]
    shas = {}
    for ver in ("v3", "v4"):
        try:
            uops = lower(spec, ver=ver)
            shas[ver] = DveOpSpec(
                name=name, opcode=row, uops=uops, rd1_en=_has_src1(spec)
            ).sha(ver)
        except Exception:
            if ver == "v3":
                raise
    op = DveOp(name, spec, subdim=False, uops_sha=shas)
    dve_ops.OPS.append(op)
    dve_ops.CUSTOM_DVE_SPECS[name] = op.spec
    _REGISTERED[name] = op
    return op


# w64 = (T*c0 + c0) + clip(T*c1 + c2, 0, 1) ; c0=31.5 c1=3.5 c2=0.5
BC6W64 = _register(
    "BC6W64_ANT",
    Spec(
        body=(Src0 * C0 + C0) + minn(relu(Src0 * C1 + C2), One),
        reference=lambda in0, in1, c0, c1, c2: (
            (in0.astype(np.float32) * c0 + c0)
            + np.minimum(
                np.maximum(in0.astype(np.float32) * c1 + c2, 0.0), 1.0
            )
        ).astype(np.float32),
    ),
)

# (Src0 - Src1) * c0  — endpoint-combo with the scale folded in
SUBSCALE = _register(
    "SUBSCALE_ANT",
    Spec(
        body=(Src0 - Src1) * C0,
        reference=lambda in0, in1, c0, c1, c2: (
            (in0.astype(np.float32) - in1.astype(np.float32)) * c0
        ).astype(np.float32),
    ),
)

# out = frac(u') * e2 = ((u'+c0) - ((u'+c1) - c2)) * Src1
FRACMUL = _register(
    "BC6FRACMUL_ANT",
    Spec(
        body=((Src0 + C0) - ((Src0 + C1) - C2)) * Src1,
        reference=lambda in0, in1, c0, c1, c2: (
            (
                (in0.astype(np.float32) + np.float32(c0)).astype(np.float32)
                - (
                    (in0.astype(np.float32) + np.float32(c1)).astype(
                        np.float32
                    )
                    - np.float32(c2)
                ).astype(np.float32)
            ).astype(np.float32)
            * in1.astype(np.float32)
        ).astype(np.float32),
    ),
)


# ------------------------------------------------------- bass kernel build
def _ap4(base, dims):
    """Manual free-dim AP: keep base's partition dim, set free dims."""
    return bass.AP(base.tensor, base.offset, [list(base.ap[0])] + dims)


def build_kernel(nb_core=NB_CORE, g=G):
    st_blocks = 128 * g
    n_st = nb_core // st_blocks
    assert nb_core % st_blocks == 0
    assert g % 4 == 0

    nc = bacc.Bacc(
        "TRN2",
        target_bir_lowering=False,
        debug=False,
        enable_asserts=False,
        num_devices=1,
    )

    ep = nc.dram_tensor("endpoints", [nb_core, 12], BF16, kind="ExternalInput").ap()
    ix = nc.dram_tensor("indices", [nb_core, 16], BF16, kind="ExternalInput").ap()
    lg = nc.dram_tensor("logits", [nb_core, 32], BF16, kind="ExternalInput").ap()
    # bank_diag: [128, 4*49] block-diagonal: row k (band q = k//32) has
    # [bank3[k%32] | 1] in cols 49q..49q+48, zeros elsewhere.  One K=128
    # matmul computes num/den for 4 groups (the 4 partition bands of one
    # transposed chunk) at once.
    bank3 = nc.dram_tensor("bank3", [128, 196], BF16, kind="ExternalInput").ap()
    ident = nc.dram_tensor("ident", [128, 128], BF16, kind="ExternalInput").ap()
    out = nc.dram_tensor("out", [nb_core, 48], BF16, kind="ExternalOutput").ap()

    with tile.TileContext(nc) as tc, ExitStack() as ctx:
        const_pool = ctx.enter_context(tc.tile_pool(name="const", bufs=1))
        in_pool = ctx.enter_context(tc.tile_pool(name="inp", bufs=4))
        mid_pool = ctx.enter_context(tc.tile_pool(name="mid", bufs=4))
        rep_pool = ctx.enter_context(tc.tile_pool(name="rep", bufs=3))
        big1_pool = ctx.enter_context(tc.tile_pool(name="big1", bufs=3))
        big2_pool = ctx.enter_context(tc.tile_pool(name="big2", bufs=3))
        out_pool = ctx.enter_context(tc.tile_pool(name="outp", bufs=4))
        ps_t = ctx.enter_context(tc.tile_pool(name="ps_t", bufs=2, space="PSUM"))
        ps_mm = ctx.enter_context(tc.tile_pool(name="ps_mm", bufs=4, space="PSUM"))

        bank_t = const_pool.tile([128, 196], BF16)
        nc.sync.dma_start(bank_t[:], bank3)
        id_t = const_pool.tile([128, 128], BF16)
        nc.sync.dma_start(id_t[:], ident)

        gh = g // 2
        for t in range(n_st):
            b0 = t * st_blocks
            # ---- loads (contiguous per partition, bf16) ----
            ep_t = in_pool.tile([128, g * 12], BF16, tag="ep")
            nc.sync.dma_start(
                ep_t[:],
                ep[b0 : b0 + st_blocks, :].rearrange("(r g) d -> r (g d)", g=g),
            )
            ix_t = in_pool.tile([128, g * 16], BF16, tag="ix")
            nc.sync.dma_start(
                ix_t[:],
                ix[b0 : b0 + st_blocks, :].rearrange("(r g) d -> r (g d)", g=g),
            )
            lg_t = in_pool.tile([128, g * 32], BF16, tag="lg")
            nc.sync.dma_start(
                lg_t[:],
                lg[b0 : b0 + st_blocks, :].rearrange("(r g) d -> r (g d)", g=g),
            )

            # ---- ACT tanh (exp_and_others set; sigmoid = (1+T)/2 folded) --
            T_t = mid_pool.tile([128, g * 12], F32, tag="tep")
            nc.scalar.activation(T_t[:], ep_t[:], AF.Tanh, bias=0.0, scale=0.5)
            W_t = mid_pool.tile([128, g * 16], F32, tag="tix")
            nc.scalar.activation(W_t[:], ix_t[:], AF.Tanh, bias=0.0, scale=0.5)

            # ---- w64 (custom DVE, one pass) ----
            w_t = mid_pool.tile([128, g * 16], F32, tag="w")
            nc.vector._custom_dve(
                BC6W64, out=w_t[:], in0=W_t[:], s0=31.5, s1=3.5, imm2=0.5
            )

            # ---- endpoint combos (small strided ops, scale pre-folded) ----
            T3v = T_t[:, :].rearrange("r (g d) -> r g d", g=g)

            def tslice(i):  # tanh of endpoint i: [128, g, 3]
                return T3v[:, :, 3 * i : 3 * i + 3]

            s2u = mid_pool.tile([128, g * 3], F32, tag="s2u")
            s2u3 = s2u[:, :].rearrange("r (g c) -> r g c", g=g)
            nc.vector.tensor_scalar(
                s2u3, tslice(2), ES2, EU_BIAS + ES2 - FLOOR_OFF_H,
                AOp.mult, AOp.add,
            )
            bus = mid_pool.tile([128, g * 3], F32, tag="bus")  # (T3-T2)*ES2/64
            bus3 = bus[:, :].rearrange("r (g c) -> r g c", g=g)
            nc.vector._custom_dve(
                SUBSCALE, out=bus3, in0=tslice(3), in1=tslice(2),
                s0=ES2 / 64.0, s1=0.0, imm2=0.0,
            )
            d02s = mid_pool.tile([128, g * 3], F32, tag="d02s")  # (T0-T2)*ES2
            d02v = d02s[:, :].rearrange("r (g c) -> r g c", g=g)
            nc.vector._custom_dve(
                SUBSCALE, out=d02v, in0=tslice(0), in1=tslice(2),
                s0=ES2, s1=0.0, imm2=0.0,
            )
            d13s = mid_pool.tile([128, g * 3], F32, tag="d13s")  # (T1-T3)*ES2
            d13v = d13s[:, :].rearrange("r (g c) -> r g c", g=g)
            nc.vector._custom_dve(
                SUBSCALE, out=d13v, in0=tslice(1), in1=tslice(3),
                s0=ES2, s1=0.0, imm2=0.0,
            )
            dds = mid_pool.tile([128, g * 3], F32, tag="dds")  # (d13s-d02s)/64
            ddv = dds[:, :].rearrange("r (g c) -> r g c", g=g)
            nc.vector._custom_dve(
                SUBSCALE, out=ddv, in0=d13v, in1=d02v,
                s0=1.0 / 64.0, s1=0.0, imm2=0.0,
            )

            # ---- logits: PE transpose (bf16) -> ACT exp -> e_T bf16 ----
            n_ch = g // 4  # chunks of 4 groups (512 blocks)
            e_T = big2_pool.tile([128, g * 32], BF16, tag="eT")
            for j in range(0, n_ch, 4):
                jn = min(4, n_ch - j)
                pst = ps_t.tile([128, 512], BF16, tag="pst")
                for q in range(jn):
                    ch = j + q
                    nc.tensor.transpose(
                        pst[:, 128 * q : 128 * (q + 1)],
                        lg_t[:, 128 * ch : 128 * (ch + 1)],
                        id_t[:],
                    )
                nc.scalar.activation(
                    e_T[:, 128 * j : 128 * (j + jn)],
                    pst[:, : 128 * jn],
                    AF.Exp,
                )

            # ---- per-chunk matmuls: [num | den] x4 groups into PSUM ----
            rcp_f = mid_pool.tile([128, g], F32, tag="rcp")
            num_tiles = []
            pmm = None
            for ch in range(n_ch):
                off = 196 * (ch % 2)
                if off == 0:
                    pmm = ps_mm.tile([128, 392], F32, tag="pmm")
                nc.tensor.matmul(
                    pmm[:, off : off + 196],
                    e_T[:, 128 * ch : 128 * (ch + 1)],
                    bank_t[:, :],
                    start=True,
                    stop=True,
                )
                if off:  # one 8-wide reciprocal per filled PSUM tile
                    nc.vector.reciprocal(
                        rcp_f[:, 4 * (ch - 1) : 4 * (ch + 1)],
                        _ap4(pmm[:, 48:], [[49, 8]]),
                    )
                num_tiles.append((ch, pmm, off))

            # ---- fold 1/den into cur/dur (scales already pre-folded) ----
            rcp_b = rcp_f[:, :].broadcast_to([128, g, 3])
            cur_f = mid_pool.tile([128, g * 3], F32, tag="cur")
            cur3 = cur_f[:, :].rearrange("r (g c) -> r g c", g=g)
            nc.vector.tensor_mul(cur3, d02v, rcp_b)
            dur_f = mid_pool.tile([128, g * 3], F32, tag="dur")
            dur3 = dur_f[:, :].rearrange("r (g c) -> r g c", g=g)
            nc.vector.tensor_mul(dur3, ddv, rcp_b)

            # ---- replicate to dense bf16 (ACT + idle GPSIMD do the
            #      broadcast reads; every assembly op below then runs in
            #      the DVE 2x bf16 perf mode) ----
            w_b = _ap4(w_t[:, :], [[16, g], [0, 3], [1, 16]])

            def cb(tile_):  # [128, g*3] -> [r, g, c, p] broadcast over p
                return tile_[:, :].rearrange("r (g c) -> r g c", g=g).broadcast_to(
                    [128, g, 3, 16]
                )

            w48 = rep_pool.tile([128, g * 48], BF16, tag="w48")
            nc.scalar.activation(w48[:], w_b, AF.Copy)
            cur48 = rep_pool.tile([128, g * 48], BF16, tag="cur48")
            nc.gpsimd.tensor_copy(cur48[:], cb(cur_f))
            dur48 = rep_pool.tile([128, g * 48], BF16, tag="dur48")
            nc.gpsimd.tensor_copy(dur48[:], cb(dur_f))
            bus48 = rep_pool.tile([128, g * 48], BF16, tag="bus48")
            nc.gpsimd.tensor_copy(bus48[:], cb(bus))
            num48 = rep_pool.tile([128, g * 48], BF16, tag="num48")
            for i in range(0, len(num_tiles), 2):              # PSUM -> bf16
                ch, pmm, off = num_tiles[i]
                npair = 8 if i + 1 < len(num_tiles) else 4
                num_b = _ap4(pmm[:, :], [[49, npair], [1, 48]])
                nc.scalar.activation(
                    num48[:, 48 * 4 * ch : 48 * 4 * ch + 48 * npair],
                    num_b,
                    AF.Copy,
                )

            # ---- assembly: u' = s2u + bus*w64 + (cur + dur*w64)*num ----
            tA = big1_pool.tile([128, g * 48], BF16, tag="tA")
            tAb = big1_pool.tile([128, g * 48], BF16, tag="tAb")
            tB = big1_pool.tile([128, g * 48], BF16, tag="tB")
            u2 = big1_pool.tile([128, g * 48], BF16, tag="u2")
            u_t = big2_pool.tile([128, g * 48], F32, tag="u")

            nc.vector.tensor_mul(tA[:], dur48[:], w48[:])      # dur*w64
            nc.vector.tensor_add(tA[:], tA[:], cur48[:])       # + cur
            nc.vector.tensor_mul(tAb[:], tA[:], num48[:])      # * num
            nc.vector.tensor_mul(tB[:], bus48[:], w48[:])      # bus*w64
            nc.vector.tensor_add(u2[:], tAb[:], tB[:])         # bf16 2x
            nc.vector.tensor_add(
                u_t[:, :].rearrange("r (g c p) -> r g c p", g=g, c=3),
                u2[:, :].rearrange("r (g c p) -> r g c p", g=g, c=3),
                cb(s2u),
            )                                                  # + s2u -> u'

            # ---- decode: out = 2^(hh-14) * (u - hh) ----
            hm = big1_pool.tile([128, g * 48], F32, tag="tA")
            nc.vector.tensor_scalar(
                hm[:], u_t[:], MAGIC, MAGIC + 14.0, AOp.add, AOp.subtract
            )
            e2_t = big1_pool.tile([128, g * 48], BF16, tag="tB")
            nc.scalar.activation(
                e2_t[:], hm[:], AF.Exp, bias=0.0, scale=LN2
            )
            o_t = out_pool.tile([128, g * 48], BF16, tag="o")
            nc.vector._custom_dve(
                FRACMUL,
                out=o_t[:],
                in0=u_t[:],
                in1=e2_t[:],
                s0=FLOOR_OFF_H,
                s1=MAGIC,
                imm2=MAGIC,
            )

            nc.sync.dma_start(
                out[b0 : b0 + st_blocks, :].rearrange("(r g) d -> r (g d)", g=g),
                o_t[:],
            )

    nc.compile()
    return nc


# ------------------------------------------------------- host-side driver
_NC_CACHE = {}


def _get_nc():
    if "nc" not in _NC_CACHE:
        _NC_CACHE["nc"] = build_kernel()
    return _NC_CACHE["nc"]


def make_in_maps(endpoints, indices, partition_logits, partition_bank, nb=NB):
    """Shard + pack host inputs into the 8 per-core input dicts."""
    b49 = np.empty((32, 49), dtype=np.float32)
    b49[:, 0:48] = np.tile(partition_bank.astype(np.float32), (1, 3)).reshape(
        32, 48
    )
    b49[:, 48] = 1.0
    bank3 = np.zeros((128, 196), dtype=np.float32)
    for q in range(4):
        bank3[32 * q : 32 * (q + 1), 49 * q : 49 * (q + 1)] = b49
    bank3 = bank3.astype(BF_NP)
    ident = np.eye(128, dtype=np.float32).astype(BF_NP)

    ep_flat = np.ascontiguousarray(
        endpoints.astype(np.float32).reshape(nb, 12)
    ).astype(BF_NP)
    ixf = np.ascontiguousarray(indices.astype(np.float32)).astype(BF_NP)
    lgf = np.ascontiguousarray(partition_logits.astype(np.float32)).astype(
        BF_NP
    )
    nbc = nb // N_CORES
    in_maps = []
    for c in range(N_CORES):
        sl = slice(c * nbc, (c + 1) * nbc)
        in_maps.append(
            {
                "endpoints": np.ascontiguousarray(ep_flat[sl]),
                "indices": np.ascontiguousarray(ixf[sl]),
                "logits": np.ascontiguousarray(lgf[sl]),
                "bank3": bank3,
                "ident": ident,
            }
        )
    return in_maps


def blocks_to_img(blocks):
    """[NB, 48] c-major blocks -> (3, H, W) image."""
    return (
        np.asarray(blocks)
        .astype(np.float32)
        .reshape(BY, BX, 3, 4, 4)
        .transpose(2, 0, 3, 1, 4)
        .reshape(3, H, W)
    )


def kernel(endpoints, indices, partition_logits, partition_bank, weight_lut):
    endpoints = np.asarray(endpoints, dtype=np.float32)
    indices = np.asarray(indices, dtype=np.float32)
    partition_logits = np.asarray(partition_logits, dtype=np.float32)
    partition_bank = np.asarray(partition_bank, dtype=np.float32)
    assert endpoints.shape[0] == NB

    in_maps = make_in_maps(endpoints, indices, partition_logits, partition_bank)
    nc = _get_nc()
    res = bass_utils.run_bass_kernel_spmd(
        nc, in_maps, core_ids=list(range(N_CORES))
    )
    blocks = np.concatenate(
        [np.asarray(res.results[c]["out"]) for c in range(N_CORES)], axis=0
    )
    return blocks_to_img(blocks)


# revision 12
# speedup vs baseline: 2.0401x; 2.0401x over previous
"""BC6H surrogate block-level decode kernel for 8 Trainium2 NeuronCores.

Full-input contract: kernel(**inputs) takes the complete arrays from
setup_inputs() and returns the full (3, 4096, 4096) image.  Internally the
block dimension (nb = 1048576) is sharded 8 ways (pure data parallel); each
core runs an identical Bass/Tile program on its 131072-block shard.

Math (per 4x4 block b, pixel p in 0..15, channel c in 0..2), with
T_i = tanh(x/2) so sigmoid(x) = (1+T)/2 and every ACT call stays in the
exp_and_others table set (no ACT table reloads):
  w64   = 31.5*T_w + 31.5 + clip(3.5*T_w + 0.5, 0, 1)        (= 64*w, exact)
  num_p = sum_k exp(l_k) bank[k,p] ; den = sum_k exp(l_k)     (TensorE, bf16)
  u'    = s2u_c + bus_c*w64 + (cur_c + dur_c*w64)*num_p
          with rcp=1/den folded into cur/dur, and s2u folding
          -(1+1/1024+0.5) so u' is the magic-round floor input
  hh-14 = (u' + M) - (M+14),  M = 1.5*2^23                    (fp32 exact)
  out   = 2^(hh-14) * ((u'+1.5009765625) - ((u'+M)-M))        (one fused op)

Engine plan (per supertile of 4096 blocks, g=32 blocks/partition-row):
  DMA   : bf16 inputs (ep/ix/lg) in, bf16 blocks out
  ACT   : tanh(ep), tanh(ix), exp(logits^T) -> bf16, exp(hh-14)
  PE    : 128x128 bf16 transposes of logits; K=128 block-diag bank matmul
  DVE   : everything elementwise (GPSIMD is 2-4x slower per trace - unused)
"""

import sys

sys.path.insert(0, "/opt/trn_rl_repo")

from contextlib import ExitStack

import numpy as np
import ml_dtypes

import concourse.bass as bass
import concourse.tile as tile
from concourse import bacc, mybir
from concourse import bass_utils
from concourse import dve_ops
from concourse.dve_ops import DveOp
from concourse.dve_spec import (
    Spec,
    Src0,
    Src1,
    C0,
    C1,
    C2,
    One,
    relu,
    minn,
    lower,
    _has_src1,
)
from concourse.dve_uop import DveOpSpec

F32 = mybir.dt.float32
BF16 = mybir.dt.bfloat16
AOp = mybir.AluOpType
AF = mybir.ActivationFunctionType
BF_NP = ml_dtypes.bfloat16

# ---------------------------------------------------------------- constants
NB = 1048576
N_CORES = 8
NB_CORE = NB // N_CORES            # 131072 blocks per core
G = 32                             # blocks per partition-row per supertile
H = W = 4096
BY = BX = 1024

EU_SCALE = 31248.0 / 1024.0        # 30.515625
EU_BIAS = 248.0 / 1024.0           # 0.2421875
ES2 = EU_SCALE / 2.0               # tanh-domain endpoint scale
FLOOR_OFF_H = 1.0 + 1.0 / 1024.0 + 0.5   # 1.5009765625 (exact in f32)
MAGIC = 12582912.0                 # 1.5 * 2^23
LN2 = 0.6931471805599453

# ------------------------------------------------------- custom DVE ops
_REGISTERED = {}


def _register(name, spec):
    if name in _REGISTERED:
        return _REGISTERED[name]
    if name not in dve_ops._SUB_OPCODE_FOR_NAME:
        row = max(dve_ops._SUB_OPCODE_FOR_NAME.values()) + 1
        assert row < 0x20, "custom-DVE opcode rows exhausted"
        dve_ops._SUB_OPCODE_FOR_NAME[name] = row
    row = dve_ops._SUB_OPCODE_FOR_NAME[name]
    shas = {}
    for ver in ("v3", "v4"):
        try:
            uops = lower(spec, ver=ver)
            shas[ver] = DveOpSpec(
                name=name, opcode=row, uops=uops, rd1_en=_has_src1(spec)
            ).sha(ver)
        except Exception:
            if ver == "v3":
                raise
    op = DveOp(name, spec, subdim=False, uops_sha=shas)
    dve_ops.OPS.append(op)
    dve_ops.CUSTOM_DVE_SPECS[name] = op.spec
    _REGISTERED[name] = op
    return op


# w64 = (T*c0 + c0) + clip(T*c1 + c2, 0, 1) ; c0=31.5 c1=3.5 c2=0.5
BC6W64 = _register(
    "BC6W64_ANT",
    Spec(
        body=(Src0 * C0 + C0) + minn(relu(Src0 * C1 + C2), One),
        reference=lambda in0, in1, c0, c1, c2: (
            (in0.astype(np.float32) * c0 + c0)
            + np.minimum(
                np.maximum(in0.astype(np.float32) * c1 + c2, 0.0), 1.0
            )
        ).astype(np.float32),
    ),
)

# (Src0 - Src1) * c0  — endpoint-combo with the scale folded in
SUBSCALE = _register(
    "SUBSCALE_ANT",
    Spec(
        body=(Src0 - Src1) * C0,
        reference=lambda in0, in1, c0, c1, c2: (
            (in0.astype(np.float32) - in1.astype(np.float32)) * c0
        ).astype(np.float32),
    ),
)

# out = frac(u') * e2 = ((u'+c0) - ((u'+c1) - c2)) * Src1
FRACMUL = _register(
    "BC6FRACMUL_ANT",
    Spec(
        body=((Src0 + C0) - ((Src0 + C1) - C2)) * Src1,
        reference=lambda in0, in1, c0, c1, c2: (
            (
                (in0.astype(np.float32) + np.float32(c0)).astype(np.float32)
                - (
                    (in0.astype(np.float32) + np.float32(c1)).astype(
                        np.float32
                    )
                    - np.float32(c2)
                ).astype(np.float32)
            ).astype(np.float32)
            * in1.astype(np.float32)
        ).astype(np.float32),
    ),
)


# ------------------------------------------------------- bass kernel build
def _ap4(base, dims):
    """Manual free-dim AP: keep base's partition dim, set free dims."""
    return bass.AP(base.tensor, base.offset, [list(base.ap[0])] + dims)


def build_kernel(nb_core=NB_CORE, g=G):
    st_blocks = 128 * g
    n_st = nb_core // st_blocks
    assert nb_core % st_blocks == 0
    assert g % 4 == 0

    nc = bacc.Bacc(
        "TRN2",
        target_bir_lowering=False,
        debug=False,
        enable_asserts=False,
        num_devices=1,
    )

    ep = nc.dram_tensor("endpoints", [nb_core, 12], BF16, kind="ExternalInput").ap()
    ix = nc.dram_tensor("indices", [nb_core, 16], BF16, kind="ExternalInput").ap()
    lg = nc.dram_tensor("logits", [nb_core, 32], BF16, kind="ExternalInput").ap()
    # bank_diag: [128, 4*49] block-diagonal: row k (band q = k//32) has
    # [bank3[k%32] | 1] in cols 49q..49q+48, zeros elsewhere.  One K=128
    # matmul computes num/den for 4 groups (the 4 partition bands of one
    # transposed chunk) at once.
    bank3 = nc.dram_tensor("bank3", [128, 196], BF16, kind="ExternalInput").ap()
    ident = nc.dram_tensor("ident", [128, 128], BF16, kind="ExternalInput").ap()
    out = nc.dram_tensor("out", [nb_core, 48], BF16, kind="ExternalOutput").ap()

    with tile.TileContext(nc) as tc, ExitStack() as ctx:
        const_pool = ctx.enter_context(tc.tile_pool(name="const", bufs=1))
        in_pool = ctx.enter_context(tc.tile_pool(name="inp", bufs=4))
        mid_pool = ctx.enter_context(tc.tile_pool(name="mid", bufs=4))
        rep_pool = ctx.enter_context(tc.tile_pool(name="rep", bufs=3))
        big1_pool = ctx.enter_context(tc.tile_pool(name="big1", bufs=3))
        big2_pool = ctx.enter_context(tc.tile_pool(name="big2", bufs=3))
        out_pool = ctx.enter_context(tc.tile_pool(name="outp", bufs=4))
        ps_t = ctx.enter_context(tc.tile_pool(name="ps_t", bufs=2, space="PSUM"))
        ps_mm = ctx.enter_context(tc.tile_pool(name="ps_mm", bufs=4, space="PSUM"))

        bank_t = const_pool.tile([128, 196], BF16)
        nc.sync.dma_start(bank_t[:], bank3)
        id_t = const_pool.tile([128, 128], BF16)
        nc.sync.dma_start(id_t[:], ident)

        gh = g // 2
        for t in range(n_st):
            b0 = t * st_blocks
            # ---- loads (contiguous per partition, bf16) ----
            ep_t = in_pool.tile([128, g * 12], BF16, tag="ep")
            nc.sync.dma_start(
                ep_t[:],
                ep[b0 : b0 + st_blocks, :].rearrange("(r g) d -> r (g d)", g=g),
            )
            ix_t = in_pool.tile([128, g * 16], BF16, tag="ix")
            nc.sync.dma_start(
                ix_t[:],
                ix[b0 : b0 + st_blocks, :].rearrange("(r g) d -> r (g d)", g=g),
            )
            lg_t = in_pool.tile([128, g * 32], BF16, tag="lg")
            nc.sync.dma_start(
                lg_t[:],
                lg[b0 : b0 + st_blocks, :].rearrange("(r g) d -> r (g d)", g=g),
            )

            # ---- ACT tanh (exp_and_others set; sigmoid = (1+T)/2 folded) --
            T_t = mid_pool.tile([128, g * 12], F32, tag="tep")
            nc.scalar.activation(T_t[:], ep_t[:], AF.Tanh, bias=0.0, scale=0.5)
            W_t = mid_pool.tile([128, g * 16], F32, tag="tix")
            nc.scalar.activation(W_t[:], ix_t[:], AF.Tanh, bias=0.0, scale=0.5)

            # ---- w64 (custom DVE, one pass) ----
            w_t = mid_pool.tile([128, g * 16], F32, tag="w")
            nc.vector._custom_dve(
                BC6W64, out=w_t[:], in0=W_t[:], s0=31.5, s1=3.5, imm2=0.5
            )

            # ---- endpoint combos (small strided ops, scale pre-folded) ----
            T3v = T_t[:, :].rearrange("r (g d) -> r g d", g=g)

            def tslice(i):  # tanh of endpoint i: [128, g, 3]
                return T3v[:, :, 3 * i : 3 * i + 3]

            s2u = mid_pool.tile([128, g * 3], F32, tag="s2u")
            s2u3 = s2u[:, :].rearrange("r (g c) -> r g c", g=g)
            nc.vector.tensor_scalar(
                s2u3, tslice(2), ES2, EU_BIAS + ES2 - FLOOR_OFF_H,
                AOp.mult, AOp.add,
            )
            bus = mid_pool.tile([128, g * 3], F32, tag="bus")  # (T3-T2)*ES2/64
            bus3 = bus[:, :].rearrange("r (g c) -> r g c", g=g)
            nc.vector._custom_dve(
                SUBSCALE, out=bus3, in0=tslice(3), in1=tslice(2),
                s0=ES2 / 64.0, s1=0.0, imm2=0.0,
            )
            d02s = mid_pool.tile([128, g * 3], F32, tag="d02s")  # (T0-T2)*ES2
            d02v = d02s[:, :].rearrange("r (g c) -> r g c", g=g)
            nc.vector._custom_dve(
                SUBSCALE, out=d02v, in0=tslice(0), in1=tslice(2),
                s0=ES2, s1=0.0, imm2=0.0,
            )
            d13s = mid_pool.tile([128, g * 3], F32, tag="d13s")  # (T1-T3)*ES2
            d13v = d13s[:, :].rearrange("r (g c) -> r g c", g=g)
            nc.vector._custom_dve(
                SUBSCALE, out=d13v, in0=tslice(1), in1=tslice(3),
                s0=ES2, s1=0.0, imm2=0.0,
            )
            dds = mid_pool.tile([128, g * 3], F32, tag="dds")  # (d13s-d02s)/64
            ddv = dds[:, :].rearrange("r (g c) -> r g c", g=g)
            nc.vector._custom_dve(
                SUBSCALE, out=ddv, in0=d13v, in1=d02v,
                s0=1.0 / 64.0, s1=0.0, imm2=0.0,
            )

            # ---- logits: PE transpose (bf16) -> ACT exp -> e_T bf16 ----
            n_ch = g // 4  # chunks of 4 groups (512 blocks)
            e_T = big2_pool.tile([128, g * 32], BF16, tag="eT")
            for j in range(0, n_ch, 4):
                jn = min(4, n_ch - j)
                pst = ps_t.tile([128, 512], BF16, tag="pst")
                for q in range(jn):
                    ch = j + q
                    nc.tensor.transpose(
                        pst[:, 128 * q : 128 * (q + 1)],
                        lg_t[:, 128 * ch : 128 * (ch + 1)],
                        id_t[:],
                    )
                nc.scalar.activation(
                    e_T[:, 128 * j : 128 * (j + jn)],
                    pst[:, : 128 * jn],
                    AF.Exp,
                )

            # ---- per-chunk matmuls: [num | den] x4 groups into PSUM ----
            rcp_f = mid_pool.tile([128, g], F32, tag="rcp")
            num_tiles = []
            pmm = None
            for ch in range(n_ch):
                off = 196 * (ch % 2)
                if off == 0:
                    pmm = ps_mm.tile([128, 392], F32, tag="pmm")
                nc.tensor.matmul(
                    pmm[:, off : off + 196],
                    e_T[:, 128 * ch : 128 * (ch + 1)],
                    bank_t[:, :],
                    start=True,
                    stop=True,
                )
                if off:  # one 8-wide reciprocal per filled PSUM tile
                    nc.vector.reciprocal(
                        rcp_f[:, 4 * (ch - 1) : 4 * (ch + 1)],
                        _ap4(pmm[:, 48:], [[49, 8]]),
                    )
                num_tiles.append((ch, pmm, off))

            # ---- fold 1/den into cur/dur (scales already pre-folded) ----
            rcp_b = rcp_f[:, :].broadcast_to([128, g, 3])
            cur_f = mid_pool.tile([128, g * 3], F32, tag="cur")
            cur3 = cur_f[:, :].rearrange("r (g c) -> r g c", g=g)
            nc.vector.tensor_mul(cur3, d02v, rcp_b)
            dur_f = mid_pool.tile([128, g * 3], F32, tag="dur")
            dur3 = dur_f[:, :].rearrange("r (g c) -> r g c", g=g)
            nc.vector.tensor_mul(dur3, ddv, rcp_b)

            # ---- replicate to dense bf16 (ACT + idle GPSIMD do the
            #      broadcast reads; every assembly op below then runs in
            #      the DVE 2x bf16 perf mode) ----
            w_b = _ap4(w_t[:, :], [[16, g], [0, 3], [1, 16]])

            def cb(tile_):  # [128, g*3] -> [r, g, c, p] broadcast over p
                return tile_[:, :].rearrange("r (g c) -> r g c", g=g).broadcast_to(
                    [128, g, 3, 16]
                )

            w48 = rep_pool.tile([128, g * 48], BF16, tag="w48")
            nc.scalar.activation(w48[:], w_b, AF.Copy)
            cur48 = rep_pool.tile([128, g * 48], BF16, tag="cur48")
            nc.scalar.activation(cur48[:], cb(cur_f), AF.Copy)
            dur48 = rep_pool.tile([128, g * 48], BF16, tag="dur48")
            nc.scalar.activation(dur48[:], cb(dur_f), AF.Copy)
            bus48 = rep_pool.tile([128, g * 48], BF16, tag="bus48")
            nc.scalar.activation(bus48[:], cb(bus), AF.Copy)
            num48 = rep_pool.tile([128, g * 48], BF16, tag="num48")
            for i in range(0, len(num_tiles), 2):              # PSUM -> bf16
                ch, pmm, off = num_tiles[i]
                npair = 8 if i + 1 < len(num_tiles) else 4
                num_b = _ap4(pmm[:, :], [[49, npair], [1, 48]])
                nc.scalar.activation(
                    num48[:, 48 * 4 * ch : 48 * 4 * ch + 48 * npair],
                    num_b,
                    AF.Copy,
                )

            # ---- assembly: u' = s2u + bus*w64 + (cur + dur*w64)*num ----
            tA = big1_pool.tile([128, g * 48], BF16, tag="tA")
            tAb = big1_pool.tile([128, g * 48], BF16, tag="tAb")
            tB = big1_pool.tile([128, g * 48], BF16, tag="tB")
            u2 = big1_pool.tile([128, g * 48], BF16, tag="u2")
            u_t = big2_pool.tile([128, g * 48], F32, tag="u")

            nc.vector.tensor_mul(tA[:], dur48[:], w48[:])      # dur*w64
            nc.vector.tensor_add(tA[:], tA[:], cur48[:])       # + cur
            nc.vector.tensor_mul(tAb[:], tA[:], num48[:])      # * num
            nc.vector.tensor_mul(tB[:], bus48[:], w48[:])      # bus*w64
            nc.vector.tensor_add(u2[:], tAb[:], tB[:])         # bf16 2x
            nc.vector.tensor_add(
                u_t[:, :].rearrange("r (g c p) -> r g c p", g=g, c=3),
                u2[:, :].rearrange("r (g c p) -> r g c p", g=g, c=3),
                cb(s2u),
            )                                                  # + s2u -> u'

            # ---- decode: out = 2^(hh-14) * (u - hh) ----
            hm = big1_pool.tile([128, g * 48], F32, tag="tA")
            nc.vector.tensor_scalar(
                hm[:], u_t[:], MAGIC, MAGIC + 14.0, AOp.add, AOp.subtract
            )
            e2_t = big1_pool.tile([128, g * 48], BF16, tag="tB")
            nc.scalar.activation(
                e2_t[:], hm[:], AF.Exp, bias=0.0, scale=LN2
            )
            o_t = out_pool.tile([128, g * 48], BF16, tag="o")
            nc.vector._custom_dve(
                FRACMUL,
                out=o_t[:],
                in0=u_t[:],
                in1=e2_t[:],
                s0=FLOOR_OFF_H,
                s1=MAGIC,
                imm2=MAGIC,
            )

            nc.sync.dma_start(
                out[b0 : b0 + st_blocks, :].rearrange("(r g) d -> r (g d)", g=g),
                o_t[:],
            )

    nc.compile()
    return nc


# ------------------------------------------------------- host-side driver
_NC_CACHE = {}


def _get_nc():
    if "nc" not in _NC_CACHE:
        _NC_CACHE["nc"] = build_kernel()
    return _NC_CACHE["nc"]


def make_in_maps(endpoints, indices, partition_logits, partition_bank, nb=NB):
    """Shard + pack host inputs into the 8 per-core input dicts."""
    b49 = np.empty((32, 49), dtype=np.float32)
    b49[:, 0:48] = np.tile(partition_bank.astype(np.float32), (1, 3)).reshape(
        32, 48
    )
    b49[:, 48] = 1.0
    bank3 = np.zeros((128, 196), dtype=np.float32)
    for q in range(4):
        bank3[32 * q : 32 * (q + 1), 49 * q : 49 * (q + 1)] = b49
    bank3 = bank3.astype(BF_NP)
    ident = np.eye(128, dtype=np.float32).astype(BF_NP)

    ep_flat = np.ascontiguousarray(
        endpoints.astype(np.float32).reshape(nb, 12)
    ).astype(BF_NP)
    ixf = np.ascontiguousarray(indices.astype(np.float32)).astype(BF_NP)
    lgf = np.ascontiguousarray(partition_logits.astype(np.float32)).astype(
        BF_NP
    )
    nbc = nb // N_CORES
    in_maps = []
    for c in range(N_CORES):
        sl = slice(c * nbc, (c + 1) * nbc)
        in_maps.append(
            {
                "endpoints": np.ascontiguousarray(ep_flat[sl]),
                "indices": np.ascontiguousarray(ixf[sl]),
                "logits": np.ascontiguousarray(lgf[sl]),
                "bank3": bank3,
                "ident": ident,
            }
        )
    return in_maps


def blocks_to_img(blocks):
    """[NB, 48] c-major blocks -> (3, H, W) image."""
    return (
        np.asarray(blocks)
        .astype(np.float32)
        .reshape(BY, BX, 3, 4, 4)
        .transpose(2, 0, 3, 1, 4)
        .reshape(3, H, W)
    )


def kernel(endpoints, indices, partition_logits, partition_bank, weight_lut):
    endpoints = np.asarray(endpoints, dtype=np.float32)
    indices = np.asarray(indices, dtype=np.float32)
    partition_logits = np.asarray(partition_logits, dtype=np.float32)
    partition_bank = np.asarray(partition_bank, dtype=np.float32)
    assert endpoints.shape[0] == NB

    in_maps = make_in_maps(endpoints, indices, partition_logits, partition_bank)
    nc = _get_nc()
    res = bass_utils.run_bass_kernel_spmd(
        nc, in_maps, core_ids=list(range(N_CORES))
    )
    blocks = np.concatenate(
        [np.asarray(res.results[c]["out"]) for c in range(N_CORES)], axis=0
    )
    return blocks_to_img(blocks)
